# revision 19
# baseline (speedup 1.0000x reference)
"""Trainium2 Bass kernel for nn_MultiHeadAttention (B=64, T=512, D=64, H=8, hs=8).

Strategy: data-parallel over batch across 8 NeuronCores (8 batches/core).
Per core, per batch (all layouts transposed: feature-major [d, t]):
  - host sends xT augmented with a ones row: xaug [65, 512]
  - QK projection as one set of matmuls producing Q^T / K^T in a "scatter"
    layout: head j of group g lives at partitions 32j..32j+8, enabling
    4-way tile_position row-packing of the per-head score matmuls.
  - scoresT[s_chunk, t] = K^T_chunk.T @ Q^T (contraction over head dim 8),
    only the causal t-range is computed; exp on ScalarE over 4 heads per
    instruction; causal diagonal 128x128 block masked by a 0/1 triangle
    multiply on VectorE.
  - AV matmuls col-packed 4 heads/bank; lhsT = [V | 1] columns so row 8 of
    each output accumulates the softmax denominator for free (V = Q: the
    reference reuses the Query projection for V).
  - reciprocal of denominators -> replicated across partitions via a
    selector matmul -> normalize -> output projection (Pw^T packed to the
    same scatter layout; garbage partitions get zero weights).
Output is written feature-major [64, 512] per batch; host transposes back.
"""

import os

import numpy as np

B, T, D, H, HS = 64, 512, 64, 8, 8
NCORES = 8
BPC = B // NCORES  # batches per core

# matmul operand mode: "f32r" (fp32 storage, float32r matmul), "f32", "bf16"
MODE = os.environ.get("KMODE", "bf16")
KNCORES = int(os.environ.get("KNCORES", "0")) or NCORES  # for debugging
KBPC = int(os.environ.get("KBPC", "0")) or BPC

_CACHE = {}


def _build(mode):
    import concourse.bacc as bacc
    import concourse.tile as tile
    from concourse import mybir

    f32 = mybir.dt.float32
    if mode == "bf16":
        sdt = mybir.dt.bfloat16  # storage dtype for matmul operand tiles

        def mm(ap):
            return ap
    elif mode == "f32r":
        sdt = f32

        def mm(ap):
            return ap.bitcast(mybir.dt.float32r)
    else:
        sdt = f32

        def mm(ap):
            return ap

    nc = bacc.Bacc("TRN2", target_bir_lowering=False, debug=False,
                   num_devices=KNCORES)

    xt_d = nc.dram_tensor("xtaug", [KBPC, 65, T], sdt, kind="ExternalInput").ap()
    wk_d = nc.dram_tensor("wk", [2, 65, 128], sdt, kind="ExternalInput").ap()
    wq_d = nc.dram_tensor("wq", [2, 65, 128], sdt, kind="ExternalInput").ap()
    qrhs_d = nc.dram_tensor("qrhs", [65, 72], sdt, kind="ExternalInput").ap()
    oc_d = nc.dram_tensor("ocol", [1, 128], sdt, kind="ExternalInput").ap()
    tri_d = nc.dram_tensor("tri", [128, 128], sdt, kind="ExternalInput").ap()
    sel_d = nc.dram_tensor("sel", [128, 128], f32, kind="ExternalInput").ap()
    pwt_d = nc.dram_tensor("pwt", [128, 128], sdt, kind="ExternalInput").ap()
    pb_d = nc.dram_tensor("pb", [64, 1], f32, kind="ExternalInput").ap()
    out_d = nc.dram_tensor("out", [KBPC, 64, T], f32, kind="ExternalOutput").ap()

    EXP = mybir.ActivationFunctionType.Exp

    with tile.TileContext(nc) as tc:
        with (
            tc.tile_pool(name="consts", bufs=1) as cp,
            tc.tile_pool(name="xa", bufs=2) as xap,
            tc.tile_pool(name="kq", bufs=2) as kqp,
            tc.tile_pool(name="qrow", bufs=2) as qrp,
            tc.tile_pool(name="exps", bufs=12) as exp_pool,
            tc.tile_pool(name="repn", bufs=2) as repp,
            tc.tile_pool(name="outs", bufs=2) as outp_pool,
            tc.tile_pool(name="ps", bufs=1, space="PSUM") as ps,
        ):
            # constants, loaded once
            wk = [cp.tile([65, 128], sdt, tag=f"wk{g}", name=f"wk{g}") for g in range(2)]
            wq = [cp.tile([65, 128], sdt, tag=f"wq{g}", name=f"wq{g}") for g in range(2)]
            qrhs = cp.tile([65, 72], sdt, tag="qrhs")
            ocol = cp.tile([1, 128], sdt, tag="ocol")
            tri = cp.tile([128, 128], sdt, tag="tri")
            sel = cp.tile([128, 128], f32, tag="sel")
            pwt = cp.tile([128, 128], sdt, tag="pwt")
            pb = cp.tile([64, 1], f32, tag="pb")
            onesrow = cp.tile([1, 512], sdt, tag="onesrow")
            nc.vector.memset(onesrow[:], 1.0)
            for g in range(2):
                nc.sync.dma_start(wk[g][:], wk_d[g])
                nc.sync.dma_start(wq[g][:], wq_d[g])
            nc.sync.dma_start(qrhs[:], qrhs_d)
            nc.sync.dma_start(ocol[:], oc_d)
            nc.sync.dma_start(tri[:], tri_d)
            nc.sync.dma_start(sel[:], sel_d)
            nc.sync.dma_start(pwt[:], pwt_d)
            nc.sync.dma_start(pb[:], pb_d)

            # group schedule alternates the 4-bank and 2-bank PSUM slots so
            # exp(group k) overlaps the score matmuls of group k+1
            GORDER = [(0, 0), (2, 0), (0, 1), (2, 1), (1, 0), (3, 0), (1, 1), (3, 1)]

            for b in range(KBPC):
                xa = xap.tile([65, T], sdt, tag="xa")
                nc.sync.dma_start(xa[:], xt_d[b])

                # QK projection -> scatter layout [128, 512] per group
                pj = ps.tile([128, 2048], f32, tag="S4")
                for i, w in enumerate(wk + wq):
                    nc.tensor.matmul(pj[:, 512 * i:512 * i + 512],
                                     mm(w[:]), mm(xa[:]), start=True, stop=True)
                ksb = kqp.tile([128, 1024], sdt, tag="k")
                nc.vector.tensor_copy(ksb[:], pj[:, 0:1024])
                qsb = kqp.tile([128, 1024], sdt, tag="q")
                nc.vector.tensor_copy(qsb[:], pj[:, 1024:2048])

                # Q rows layout [t, (h, e | 1)] for AV lhsT (V = Q)
                qr = ps.tile([128, 288], f32, tag="S2")
                for ci in range(4):
                    nc.tensor.matmul(qr[:, 72 * ci:72 * ci + 72],
                                     mm(xa[:, 128 * ci:128 * ci + 128]),
                                     mm(qrhs[:]), start=True, stop=True)
                qrow = qrp.tile([128, 288], sdt, tag="qrow")
                nc.vector.tensor_copy(qrow[:], qr[:])

                av = ps.tile([128, 1024], f32, tag="AV")
                # pre-fill the AV banks with ones via a rank-1 matmul (ones
                # column x the ones row of xaug): lanes never touched by the
                # col-packed AV outputs must stay finite through the
                # whole-bank reciprocal below (their products are later
                # multiplied by zero selector/projection weights)
                for g in range(2):
                    nc.tensor.matmul(av[:, 512 * g:512 * g + 512],
                                     mm(ocol[:]),
                                     mm(onesrow[:]), start=True, stop=True)

                exs = {}
                for (ci, g) in GORDER:
                    Nt = T - 128 * ci
                    # Concurrent (tile_position-packed) matmuls must each own
                    # a PSUM bank: two packed matmuls writing one bank crash
                    # the device. Wide chunks (ci<2) pack 4 heads across the
                    # 4 banks of the S4 slot; narrow chunks pack 2 heads
                    # across the 2 banks of the S2 slot, twice, with the
                    # pair's exp read forcing serialization in between.
                    if ci < 2:
                        S = ps.tile([128, 2048], f32, tag="S4")
                        for j in range(4):
                            nc.tensor.matmul(
                                S[:, 512 * j:512 * j + Nt],
                                mm(ksb[32 * j:32 * j + 8,
                                       512 * g + 128 * ci:512 * g + 128 * ci + 128]),
                                mm(qsb[32 * j:32 * j + 8,
                                       512 * g + 128 * ci:512 * g + 512]),
                                start=True, stop=True, tile_position=(32 * j, 0))
                        ex = exp_pool.tile([128, 2048], sdt, tag="ex")
                        in_ap = S[:, 0:2048].rearrange("p (j n) -> p j n", j=4)[:, :, 0:Nt]
                        out_ap = ex[:, 0:4 * Nt].rearrange("p (j n) -> p j n", j=4)
                        nc.scalar.activation(out_ap, in_ap, EXP)
                    else:
                        S = ps.tile([128, 1024], f32, tag="S2")
                        ex = exp_pool.tile([128, 2048], sdt, tag="ex")
                        for pair in range(2):
                            for idx in range(2):
                                j = 2 * pair + idx
                                nc.tensor.matmul(
                                    S[:, 512 * idx:512 * idx + Nt],
                                    mm(ksb[32 * j:32 * j + 8,
                                           512 * g + 128 * ci:512 * g + 128 * ci + 128]),
                                    mm(qsb[32 * j:32 * j + 8,
                                           512 * g + 128 * ci:512 * g + 512]),
                                    start=True, stop=True,
                                    tile_position=(32 * j, 0))
                            in_ap = S[:, 0:1024].rearrange(
                                "p (i n) -> p i n", i=2)[:, :, 0:Nt]
                            out_ap = ex[:, 2 * Nt * pair:2 * Nt * (pair + 1)
                                        ].rearrange("p (i n) -> p i n", i=2)
                            nc.scalar.activation(out_ap, in_ap, EXP)
                    # causal mask on the diagonal 128-wide block of each head
                    dg = ex[:, 0:4 * Nt].rearrange("p (j n) -> p j n", j=4)[:, :, 0:128]
                    nc.vector.tensor_mul(
                        dg, dg, tri[:].unsqueeze(1).broadcast_to([128, 4, 128]))
                    exs[(ci, g)] = ex

                # AV accumulation: each (g, j) head's 4-matmul accumulation
                # group is emitted consecutively so groups touching the same
                # PSUM bank never interleave their start/stop windows
                for g in range(2):
                    for j in range(4):
                        h = 4 * g + j
                        for ci in range(4):
                            Nt = T - 128 * ci
                            nc.tensor.matmul(
                                av[32 * j:32 * j + 9,
                                   512 * g + 128 * ci:512 * g + 512],
                                mm(qrow[:, 72 * ci + 9 * h:72 * ci + 9 * h + 9]),
                                mm(exs[(ci, g)][:, Nt * j:Nt * j + Nt]),
                                start=(ci == 0), stop=(ci == 3),
                                tile_position=(0, 32 * j),
                                skip_group_check=True)

                # reciprocal of the whole AV block (denominators live at
                # partitions 32j+8; other lanes are finite and later killed by
                # zero weights), then replicate across each 32-strip via the
                # selector matmul in plain fp32 (0/1 weights, exact)
                rspf = repp.tile([128, 1024], f32, tag="rspf")
                nc.vector.reciprocal(rspf[:], av[:])
                rp = ps.tile([128, 1024], f32, tag="S2")
                for g in range(2):
                    nc.tensor.matmul(rp[:, 512 * g:512 * g + 512], sel[:],
                                     rspf[:, 512 * g:512 * g + 512],
                                     start=True, stop=True)
                repsb = repp.tile([128, 1024], f32, tag="rep")
                nc.vector.tensor_copy(repsb[:], rp[:])
                nrm = repp.tile([128, 1024], sdt, tag="nrm")
                nc.vector.tensor_mul(nrm[:], av[:], repsb[:])

                op = ps.tile([64, 512], f32, tag="S2")
                for g in range(2):
                    nc.tensor.matmul(op[:], mm(pwt[:, 64 * g:64 * g + 64]),
                                     mm(nrm[:, 512 * g:512 * g + 512]),
                                     start=(g == 0), stop=(g == 1))
                of = outp_pool.tile([64, 512], f32, tag="of")
                nc.vector.tensor_scalar_add(of[:], op[:], pb[:])
                nc.sync.dma_start(out_d[b], of[:])

    nc.compile()
    return nc


def _get(mode):
    if mode not in _CACHE:
        _CACHE[mode] = _build(mode)
    return _CACHE[mode]


def _np_dt(mode):
    if mode == "bf16":
        import ml_dtypes
        return np.dtype(ml_dtypes.bfloat16)
    return np.dtype(np.float32)


def _pack_weights(Kw, Kb, Qw, Qb, Pw, Pb, mode):
    dt = _np_dt(mode)
    rs = np.float32(1.0 / np.sqrt(HS))
    Kw_s, Kb_s = Kw * rs, Kb * rs
    wk = np.zeros((2, 65, 128), np.float32)
    wq = np.zeros((2, 65, 128), np.float32)
    for g in range(2):
        for j in range(4):
            h = 4 * g + j
            wk[g, 0:64, 32 * j:32 * j + 8] = Kw_s[h]
            wk[g, 64, 32 * j:32 * j + 8] = Kb_s[h]
            wq[g, 0:64, 32 * j:32 * j + 8] = Qw[h]
            wq[g, 64, 32 * j:32 * j + 8] = Qb[h]
    qrhs = np.zeros((65, 72), np.float32)
    for h in range(H):
        qrhs[0:64, 9 * h:9 * h + 8] = Qw[h]
        qrhs[64, 9 * h:9 * h + 8] = Qb[h]
        qrhs[64, 9 * h + 8] = 1.0
    tri = (np.arange(128)[None, :] >= np.arange(128)[:, None]).astype(np.float32)
    sel = np.zeros((128, 128), np.float32)
    for m in range(128):
        sel[32 * (m // 32) + 8, m] = 1.0
    pwt = np.zeros((128, 128), np.float32)
    for g in range(2):
        for j in range(4):
            h = 4 * g + j
            for r in range(8):
                pwt[32 * j + r, 64 * g:64 * g + 64] = Pw[:, 8 * h + r]
    pb = np.ascontiguousarray(Pb.reshape(64, 1).astype(np.float32))
    return {
        "wk": wk.astype(dt), "wq": wq.astype(dt), "qrhs": qrhs.astype(dt),
        "ocol": np.ones((1, 128), np.float32).astype(dt),
        "tri": tri.astype(dt), "sel": sel.astype(np.float32),
        "pwt": pwt.astype(dt), "pb": pb,
    }


def kernel(inputs, Kw, Kb, Qw, Qb, Pw, Pb):
    from concourse.bass_utils import run_bass_kernel_spmd

    inputs = np.asarray(inputs, dtype=np.float32)
    Kw = np.asarray(Kw, np.float32); Kb = np.asarray(Kb, np.float32)
    Qw = np.asarray(Qw, np.float32); Qb = np.asarray(Qb, np.float32)
    Pw = np.asarray(Pw, np.float32); Pb = np.asarray(Pb, np.float32)

    nc = _get(MODE)
    shared = _pack_weights(Kw, Kb, Qw, Qb, Pw, Pb, MODE)
    dt = _np_dt(MODE)

    in_maps = []
    for c in range(NCORES):
        xt = np.ones((BPC, 65, T), np.float32)
        xt[:, 0:64, :] = inputs[c * BPC:(c + 1) * BPC].transpose(0, 2, 1)
        in_maps.append({"xtaug": np.ascontiguousarray(xt.astype(dt)), **shared})

    res = run_bass_kernel_spmd(nc, in_maps, list(range(NCORES)))

    out = np.empty((B, T, D), np.float32)
    for c in range(NCORES):
        out[c * BPC:(c + 1) * BPC] = res.results[c]["out"].transpose(0, 2, 1)
    return out
